# revision 1
# baseline (speedup 1.0000x reference)
"""Trainium2 Bass kernel: AdaptiveDiscretizedNeuralODE (30-step scan with
training-mode BatchNorm over the HW=1024 channel axis, ReLU6, residual).

Strategy:
 - Channel-shard the 1024 BN channels over 8 NeuronCores -> 128 channels/core
   = exactly the 128 SBUF partitions. BN stats/affine/ReLU6/residual are all
   per-channel, so cores are fully independent (no collectives).
 - Keep the whole per-core state (2MB fp32) SBUF-resident for all 30 steps.
 - Scale folding: BN is invariant under per-layer positive rescaling of its
   input (with eps adjusted by 1/alpha^2), so the recurrence
       y' = (1-dt) y + dt relu6(BN(y + m*x1))
   becomes, with yhat = y/alpha_l and ztilde_l = yhat_l + (m_l/alpha_l)*x1:
       ztilde_{l+1} = ztilde_l + min(relu(a_l ztilde_l + b_l), 6 c_l)
                      + (mtil_{l+1}-mtil_l)*x1
   where a_l, b_l fold the BN affine and the dt-step scale c_l = dt_l/alpha_{l+1}.
 - Per layer engine split:
     ACT:   u = Relu(a*z + b)          (per-partition scale/bias APs)
            Square(z_new) accum -> sum of squares (SS)
     DVE:   z += min(u, 6c)            (scalar_tensor_tensor, in-place)
            z += P_l, accum -> sum (S) (P_l = dmtil_l * x1)
            small [128,1] stats ops
     GPSIMD: P_{l+1} = x1 * dmtil_{l+1} (tensor_tensor with broadcast table)
"""
import numpy as np

B, C, H, W = 16, 256, 32, 32
HW = H * W
NL = 30
EPS = 1e-5
NCORES = 8
P = 128              # channels per core = SBUF partitions
FB = B * C           # free elements per partition = 4096
NCH = 4              # chunks per full-tile pass
CHW = FB // NCH      # 1024
CB = C               # 256 (c-block; m tables repeat with period CB)
NRED = float(FB)     # BN reduction count per channel

_cached_nc = None


def _build_program():
    import concourse.tile as tile
    from concourse import bacc, mybir

    f32 = mybir.dt.float32
    Alu = mybir.AluOpType
    Act = mybir.ActivationFunctionType

    nc = bacc.Bacc("TRN2", target_bir_lowering=False, debug=False,
                   num_devices=NCORES)
    x1_d = nc.dram_tensor("x1", [P, FB], f32, kind="ExternalInput").ap()
    mtab_d = nc.dram_tensor("mtab", [P, 31 * CB], f32, kind="ExternalInput").ap()
    ctab_d = nc.dram_tensor("ctab", [P, 3 * NL], f32, kind="ExternalInput").ap()
    ltab_d = nc.dram_tensor("ltab", [P, 2 * NL + 1], f32, kind="ExternalInput").ap()
    out_d = nc.dram_tensor("out", [P, FB], f32, kind="ExternalOutput").ap()

    with tile.TileContext(nc) as tc:
        with (
            tc.tile_pool(name="big", bufs=1) as big,
            tc.tile_pool(name="upool", bufs=2) as upool,
            tc.tile_pool(name="jpool", bufs=2) as jpool,
            tc.tile_pool(name="apool", bufs=2) as apool,
            tc.tile_pool(name="spool", bufs=3) as spool,
        ):
            zt = big.tile([P, FB], f32, name="zt")
            x1t = big.tile([P, FB], f32, name="x1t")
            pb0 = big.tile([P, FB], f32, name="pb0")
            pb1 = big.tile([P, FB], f32, name="pb1")
            pb = [pb0, pb1]
            mt = big.tile([P, 31 * CB], f32, name="mt")
            ct = big.tile([P, 3 * NL], f32, name="ct")
            lt = big.tile([P, 2 * NL + 1], f32, name="lt")

            def chs(chi):
                return slice(chi * CHW, (chi + 1) * CHW)

            def mview(idx):
                # [P, CB] table block broadcast along the b-repeat dim of a chunk
                return (mt[:, idx * CB:(idx + 1) * CB]
                        .unsqueeze(1).broadcast_to([P, CHW // CB, CB]))

            def xview(t, chi):
                return t[:, chs(chi)].rearrange("p (r c) -> p r c", c=CB)

            # ---- input DMAs (chunked so compute can start early)
            for chi in range(NCH):
                nc.sync.dma_start(x1t[:, chs(chi)], x1_d[:, chs(chi)])
            nc.sync.dma_start(mt[:, 0:2 * CB], mtab_d[:, 0:2 * CB])
            nc.sync.dma_start(ct[:], ctab_d)
            nc.sync.dma_start(lt[:], ltab_d)
            nc.sync.dma_start(mt[:, 2 * CB:], mtab_d[:, 2 * CB:])

            # ---- prologue: zt = x1*g0, S0/SS0 stats, gpsimd makes P_0
            Sacc = apool.tile([P, NCH], f32, name="Sacc_p", tag="Sacc")
            SSacc = apool.tile([P, NCH], f32, name="SSacc_p", tag="SSacc")
            for chi in range(NCH):
                nc.vector.tensor_tensor(xview(zt, chi), xview(x1t, chi),
                                        mview(0), op=Alu.mult)
                nc.vector.tensor_reduce(Sacc[:, chi:chi + 1], zt[:, chs(chi)],
                                        axis=mybir.AxisListType.X, op=Alu.add)
                jt = jpool.tile([P, CHW], f32, name=f"j_p{chi}", tag="junk")
                nc.scalar.activation(jt[:], zt[:, chs(chi)], Act.Square,
                                     bias=0.0, scale=1.0,
                                     accum_out=SSacc[:, chi:chi + 1])
                nc.gpsimd.tensor_tensor(xview(pb[0], chi), xview(x1t, chi),
                                        mview(1), op=Alu.mult)

            for l in range(NL):
                # ---- per-layer BN coefficients from S/SS (all [128,1])
                Sg = spool.tile([P, 1], f32, name=f"Sg{l}", tag="Sg")
                SSg = spool.tile([P, 1], f32, name=f"SSg{l}", tag="SSg")
                nc.vector.tensor_reduce(Sg[:], Sacc[:],
                                        axis=mybir.AxisListType.X, op=Alu.add)
                nc.vector.tensor_reduce(SSg[:], SSacc[:],
                                        axis=mybir.AxisListType.X, op=Alu.add)
                t1 = spool.tile([P, 1], f32, name=f"t1_{l}", tag="t1")
                nc.vector.tensor_scalar(t1[:], SSg[:], NRED, None, op0=Alu.mult)
                tn = spool.tile([P, 1], f32, name=f"tn{l}", tag="tn")
                nc.vector.scalar_tensor_tensor(tn[:], Sg[:], Sg[:], t1[:],
                                               op0=Alu.mult, op1=Alu.subtract)
                v = spool.tile([P, 1], f32, name=f"v{l}", tag="v")
                nc.vector.tensor_scalar(v[:], tn[:], -1.0, lt[:, l:l + 1],
                                        op0=Alu.mult, op1=Alu.add)
                rc = spool.tile([P, 1], f32, name=f"rc{l}", tag="rc")
                nc.vector.reciprocal(rc[:], v[:])
                rs = spool.tile([P, 1], f32, name=f"rs{l}", tag="rs")
                nc.scalar.activation(rs[:], rc[:], Act.Sqrt)
                a = spool.tile([P, 1], f32, name=f"a{l}", tag="a")
                nc.vector.tensor_scalar(a[:], rs[:], ct[:, l:l + 1], None,
                                        op0=Alu.mult)
                an2 = spool.tile([P, 1], f32, name=f"an2_{l}", tag="an2")
                nc.vector.tensor_scalar(an2[:], rs[:], ct[:, NL + l:NL + l + 1],
                                        None, op0=Alu.mult)
                bb = spool.tile([P, 1], f32, name=f"bb{l}", tag="bb")
                nc.vector.scalar_tensor_tensor(bb[:], Sg[:], an2[:],
                                               ct[:, 2 * NL + l:2 * NL + l + 1],
                                               op0=Alu.mult, op1=Alu.add)

                sixc_ap = lt[:, NL + l:NL + l + 1]
                if l < NL - 1:
                    Sacc = apool.tile([P, NCH], f32, name=f"Sacc{l}", tag="Sacc")
                    SSacc = apool.tile([P, NCH], f32, name=f"SSacc{l}", tag="SSacc")
                for chi in range(NCH):
                    u = upool.tile([P, CHW], f32, name=f"u{l}_{chi}", tag=f"u{chi}")
                    nc.scalar.activation(u[:], zt[:, chs(chi)], Act.Relu,
                                         bias=bb[:], scale=a[:])
                    nc.vector.scalar_tensor_tensor(zt[:, chs(chi)], u[:], sixc_ap,
                                                   zt[:, chs(chi)],
                                                   op0=Alu.min, op1=Alu.add)
                    if l < NL - 1:
                        nc.vector.scalar_tensor_tensor(
                            zt[:, chs(chi)], pb[l % 2][:, chs(chi)], 0.0,
                            zt[:, chs(chi)], op0=Alu.bypass, op1=Alu.add,
                            accum_out=Sacc[:, chi:chi + 1])
                        jt = jpool.tile([P, CHW], f32, name=f"j{l}_{chi}", tag="junk")
                        nc.scalar.activation(jt[:], zt[:, chs(chi)], Act.Square,
                                             bias=0.0, scale=1.0,
                                             accum_out=SSacc[:, chi:chi + 1])
                        if l < NL - 2:
                            # produce P_{l+1} = dmtil_{l+1} * x1 (table block l+2)
                            nc.gpsimd.tensor_tensor(xview(pb[(l + 1) % 2], chi),
                                                    xview(x1t, chi),
                                                    mview(l + 2), op=Alu.mult)
                        elif l == NL - 2:
                            # produce the epilogue tile gfin * x1 (table block 30)
                            nc.gpsimd.tensor_tensor(xview(pb[(l + 1) % 2], chi),
                                                    xview(x1t, chi),
                                                    mview(30), op=Alu.mult)

            # ---- epilogue: out = alpha_L * zt + gfin * x1  (gfin*x1 in pb[1])
            alpha_ap = lt[:, 2 * NL:2 * NL + 1]
            for chi in range(NCH):
                o = upool.tile([P, CHW], f32, name=f"o{chi}", tag=f"u{chi}")
                nc.vector.scalar_tensor_tensor(o[:], zt[:, chs(chi)], alpha_ap,
                                               pb[1][:, chs(chi)],
                                               op0=Alu.mult, op1=Alu.add)
                nc.sync.dma_start(out_d[:, chs(chi)], o[:])

    nc.compile()
    return nc


def _get_nc():
    global _cached_nc
    if _cached_nc is None:
        _cached_nc = _build_program()
    return _cached_nc


def _prepare_in_maps(x, delta_t, matrices, gamma, beta):
    dt = np.clip(delta_t.astype(np.float64), 0, 6)[:, 0]
    m = matrices.reshape(NL, C).astype(np.float64)
    alpha = np.concatenate([[1.0], np.cumprod(1.0 - dt)])
    mtil = m / alpha[:NL, None]
    cc = dt / alpha[1:]
    g0 = 1.0 + mtil[0]
    dmt = mtil[1:] - mtil[:-1]                     # [29, 256]
    gfin = 1.0 - alpha[NL] * mtil[NL - 1]
    epst = EPS / alpha[:NL] ** 2

    mtab = np.concatenate([g0[None], dmt, gfin[None]], axis=0)   # [31, 256]
    mtab_b = np.broadcast_to(mtab.reshape(1, 31 * C).astype(np.float32),
                             (P, 31 * C)).copy()

    n2eps = NRED * NRED * epst
    sixc = 6.0 * cc
    ltab_row = np.concatenate([n2eps, sixc, [alpha[NL]]]).astype(np.float32)
    ltab_b = np.broadcast_to(ltab_row, (P, ltab_row.size)).copy()

    g64 = gamma.astype(np.float64)
    b64 = beta.astype(np.float64)
    x1_full = x.reshape(B, C, HW).transpose(2, 0, 1)   # [HW, B, C]

    in_maps = []
    for k in range(NCORES):
        sl = slice(k * P, (k + 1) * P)
        cgN = (cc[:, None] * g64[None, sl] * NRED).T.astype(np.float32)
        cgneg = (-cc[:, None] * g64[None, sl]).T.astype(np.float32)
        cb = (cc[:, None] * b64[None, sl]).T.astype(np.float32)
        ctab = np.ascontiguousarray(np.concatenate([cgN, cgneg, cb], axis=1))
        x1s = np.ascontiguousarray(x1_full[sl]).reshape(P, FB).astype(np.float32)
        in_maps.append({"x1": x1s, "mtab": mtab_b, "ctab": ctab, "ltab": ltab_b})
    return in_maps


def _gather(results):
    out = np.empty((HW, B, C), dtype=np.float32)
    for k in range(NCORES):
        out[k * P:(k + 1) * P] = results[k]["out"].reshape(P, B, C)
    return np.ascontiguousarray(out.transpose(1, 2, 0).reshape(B, C, H, W))


def _run(trace, **inputs):
    from concourse.bass_utils import run_bass_kernel_spmd
    nc = _get_nc()
    in_maps = _prepare_in_maps(
        np.asarray(inputs["x"]), np.asarray(inputs["delta_t"]),
        np.asarray(inputs["matrices"]), np.asarray(inputs["gamma"]),
        np.asarray(inputs["beta"]))
    res = run_bass_kernel_spmd(nc, in_maps, core_ids=list(range(NCORES)),
                               trace=trace)
    return _gather(res.results), res


def kernel(**inputs) -> np.ndarray:
    out, _ = _run(False, **inputs)
    return out


def kernel_traced(**inputs):
    """Returns (output, BassKernelResults) with exec_time_ns populated."""
    return _run(True, **inputs)
